# revision 7
# baseline (speedup 1.0000x reference)
"""Trainium2 Bass kernel for attention pooling.

Computation (per reference):
    proj = tanh(h @ W.T + b)            # [B, L, D]
    scores = proj @ ctx                 # [B, L]
    alpha = softmax(scores, axis=L)     # [B, L]
    wo = sum_l alpha[b,l] * h[b,l,:]    # [B, D]
    returns (wo, alpha)

Sharding: data-parallel over batch across 8 NeuronCores (8 batches/core).
Host pre-transposes h and W so the device does zero layout transposes:
  - xt = h_core.T  [D, T]  feeds the projection matmul (contraction dim d
    must live on SBUF partitions for the PE).
  - x  = h_core    [T, D]  natural layout feeds the weighted sum
    (contraction dim l on partitions).

Per-core device pipeline (all static/unrolled, Tile framework):
  proj.T[e,t] accumulated in PSUM over 6 d-chunks (float32r matmuls,
  full PE rate at N=512), tanh+bias fused on ScalarE, scores via M=1
  matmuls (ctx as stationary), softmax on DVE/ACT in 2-batch groups so
  the PE never idles long, weighted sum via M=1 matmuls from natural x.
"""

import numpy as np

B, L, D = 64, 512, 768
NCORES = 8
BLOC = B // NCORES          # 8 batches per core
T = BLOC * L                # 4096 tokens per core
NDT = D // 128              # 6 contraction chunks
NET = D // 128              # 6 output-feature chunks
NL = L // 128               # 4 l-chunks per batch
DH = D // 2                 # 384, n-split for the weighted sum
GROUP = 2                   # batches per softmax group
NGROUPS = BLOC // GROUP

# Main matmul dtype: float32r streams fp32 through the PE at full
# (bf16) rate for free dims >= 256.  Fallback to mybir.dt.float32
# (4 cycles/row) if hardware precision turns out insufficient.
USE_F32R = True

_cached_nc = None


def _legalize_waits(nc, mybir):
    """This walrus build encodes at most ONE semaphore wait per instruction
    (and zero on float32r matmuls, whose waits land on the LDWEIGHTS
    struct). Tile emits correct-but-denser sync_info; split excess waits
    onto same-engine NoOps inserted immediately before the instruction."""
    import bass_rust

    k = 0
    for fn in nc.m.functions:
        for bb in fn.blocks:
            insts = bb.instructions
            out = []
            changed = False
            for inst in insts:
                si = inst.sync_info
                waits = list(si.on_wait) if (si and si.on_wait) else []
                max_waits = 0 if inst.opcode == "Matmult" else 1
                if len(waits) > max_waits:
                    keep = waits[len(waits) - max_waits :] if max_waits else []
                    spill = waits[: len(waits) - max_waits]
                    for w in spill:
                        nop = mybir.InstNoOp(
                            name=f"{inst.name}-w{k}", engine=inst.engine
                        )
                        k += 1
                        nop.sync_info = bass_rust.SyncInfo(
                            on_wait=[w], on_update=[]
                        )
                        out.append(nop)
                    inst.sync_info = bass_rust.SyncInfo(
                        on_wait=keep, on_update=list(si.on_update or [])
                    )
                    changed = True
                out.append(inst)
            if changed:
                bb.instructions = out
    return k


def _build_bass():
    import concourse.bass as bass
    import concourse.mybir as mybir
    import concourse.tile as tile
    from concourse.masks import make_identity

    f32 = mybir.dt.float32
    fr = mybir.dt.float32r if USE_F32R else mybir.dt.float32

    def mm(x):
        return x

    nc = bass.Bass("TRN2", target_bir_lowering=False, debug=False)

    x = nc.dram_tensor("x", [T, D], fr, kind="ExternalInput").ap()
    xt = nc.dram_tensor("xt", [D, T], fr, kind="ExternalInput").ap()
    wt = nc.dram_tensor("wt", [D, D], fr, kind="ExternalInput").ap()
    bias = nc.dram_tensor("bias", [D], f32, kind="ExternalInput").ap()
    ctxv = nc.dram_tensor("ctxv", [D], fr, kind="ExternalInput").ap()
    wo = nc.dram_tensor("wo", [BLOC, D], f32, kind="ExternalOutput").ap()
    aw = nc.dram_tensor("aw", [BLOC, L], f32, kind="ExternalOutput").ap()

    with tile.TileContext(nc) as tc:
        with (
            tc.tile_pool(name="consts", bufs=1) as consts,
            tc.tile_pool(name="xres", bufs=1) as xres,
            tc.tile_pool(name="xtp", bufs=2) as xtp,
            tc.tile_pool(name="projp", bufs=2) as projp,
            tc.tile_pool(name="small", bufs=1) as small,
            tc.tile_pool(name="ppp", bufs=2, space="PSUM") as ppp,
            tc.tile_pool(name="psp", bufs=2, space="PSUM") as psp,
            tc.tile_pool(name="ptp", bufs=1, space="PSUM") as ptp,
            tc.tile_pool(name="pwp", bufs=2, space="PSUM") as pwp,
        ):
            # ---- constants ----
            wt_sb = consts.tile([128, NDT, D], fr)
            nc.sync.dma_start(
                out=wt_sb, in_=wt.rearrange("(dt p) e -> p dt e", p=128)
            )
            b_sb = consts.tile([128, NET], f32)
            nc.sync.dma_start(
                out=b_sb, in_=bias.rearrange("(et p) -> p et", p=128)
            )
            c_sb = consts.tile([128, NET], fr)
            nc.sync.dma_start(
                out=c_sb, in_=ctxv.rearrange("(et p) -> p et", p=128)
            )
            ident = consts.tile([128, 128], f32)
            make_identity(nc, ident)

            # resident natural-layout activations: token t = (bb*NL + c)*128 + p
            x_sb = xres.tile([128, BLOC, NL, D], fr)
            x_re = x.rearrange("(bb c p) d -> p bb c d", p=128, c=NL)
            xt_re = xt.rearrange("(dt p) (bb t) -> p dt bb t", p=128, bb=BLOC)

            at_sb = small.tile([128, NL, BLOC], fr)   # alpha transposed
            # per-group softmax staging, all based at partition 0
            S_g = [small.tile([GROUP, L], f32, name=f"S_{g}") for g in range(NGROUPS)]
            A_g = [small.tile([GROUP, L], f32, name=f"A_{g}") for g in range(NGROUPS)]
            E_g = [small.tile([GROUP, L], f32, name=f"E_{g}") for g in range(NGROUPS)]
            nmx_g = [small.tile([GROUP, 1], f32, name=f"nmx_{g}") for g in range(NGROUPS)]
            sum_g = [small.tile([GROUP, 1], f32, name=f"sum_{g}") for g in range(NGROUPS)]
            rec_g = [small.tile([GROUP, 1], f32, name=f"rec_{g}") for g in range(NGROUPS)]

            tanh = mybir.ActivationFunctionType.Tanh
            exp = mybir.ActivationFunctionType.Exp

            def emit_batch(bb):
                g, j = divmod(bb, GROUP)
                nc.sync.dma_start(out=x_sb[:, bb, :, :], in_=x_re[:, bb, :, :])
                xt_t = xtp.tile([128, NDT, L], fr, tag="xt", name=f"xt_{bb}")
                nc.sync.dma_start(out=xt_t, in_=xt_re[:, :, bb, :])

                proj = projp.tile([128, NET, L], fr, tag="proj", name=f"proj_{bb}")
                for et in range(NET):
                    pp = ppp.tile([128, L], f32, tag="pp", name=f"pp_{bb}_{et}")
                    for dt in range(NDT):
                        nc.tensor.matmul(
                            pp,
                            lhsT=mm(wt_sb[:, dt, et * 128 : (et + 1) * 128]),
                            rhs=mm(xt_t[:, dt, :]),
                            start=(dt == 0),
                            stop=(dt == NDT - 1),
                        )
                    nc.scalar.activation(
                        out=proj[:, et, :],
                        in_=pp,
                        func=tanh,
                        bias=b_sb[:, et : et + 1],
                        scale=1.0,
                    )
                ps = psp.tile([1, L], f32, tag="ps", name=f"ps_{bb}")
                for et in range(NET):
                    nc.tensor.matmul(
                        ps,
                        lhsT=mm(c_sb[:, et : et + 1]),
                        rhs=mm(proj[:, et, :]),
                        start=(et == 0),
                        stop=(et == NET - 1),
                    )
                # PSUM has no DMA port: drain scores via DVE, then DMA the
                # row to row j of the group staging tile (DMA can retarget
                # partitions; compute engines cannot).
                srow = small.tile([1, L], f32, tag="srow", name=f"srow_{bb}", bufs=2)
                nc.vector.tensor_copy(srow, ps)
                nc.sync.dma_start(out=S_g[g][j : j + 1, :], in_=srow)

            def emit_softmax(g):
                nc.vector.reduce_max(
                    out=nmx_g[g],
                    in_=S_g[g],
                    axis=mybir.AxisListType.X,
                    negate=True,
                )
                nc.scalar.activation(
                    out=E_g[g],
                    in_=S_g[g],
                    func=exp,
                    bias=nmx_g[g],
                    scale=1.0,
                    accum_out=sum_g[g],
                )
                nc.vector.reciprocal(rec_g[g], sum_g[g])
                nc.vector.tensor_scalar_mul(A_g[g], E_g[g], rec_g[g])
                nc.sync.dma_start(
                    out=aw[g * GROUP : (g + 1) * GROUP, :], in_=A_g[g]
                )

            def emit_alpha_t(g):
                # PE work that depends on softmax(g): emitted once group g+1's
                # projection work is already queued so the PE never stalls.
                lo = g * GROUP
                for c in range(NL):
                    pt = ptp.tile([128, GROUP], f32, tag="pt", name=f"pt_{g}_{c}")
                    nc.tensor.transpose(
                        pt,
                        A_g[g][:, c * 128 : (c + 1) * 128],
                        ident[:GROUP, :GROUP],
                    )
                    nc.vector.tensor_copy(at_sb[:, c, lo : lo + GROUP], pt)

            def emit_wsum(g):
                for bb in range(g * GROUP, (g + 1) * GROUP):
                    wrow = small.tile([1, D], f32, tag="wrow", name=f"wrow_{bb}", bufs=2)
                    for h in range(2):
                        pw = pwp.tile([1, DH], f32, tag="pw", name=f"pw_{bb}_{h}")
                        for c in range(NL):
                            nc.tensor.matmul(
                                pw,
                                lhsT=mm(at_sb[:, c, bb : bb + 1]),
                                rhs=mm(x_sb[:, bb, c, h * DH : (h + 1) * DH]),
                                start=(c == 0),
                                stop=(c == NL - 1),
                            )
                        nc.vector.tensor_copy(wrow[:, h * DH : (h + 1) * DH], pw)
                    nc.sync.dma_start(out=wo[bb : bb + 1, :], in_=wrow)

            for g in range(NGROUPS):
                for bb in range(g * GROUP, (g + 1) * GROUP):
                    emit_batch(bb)
                emit_softmax(g)
                if g > 0:
                    emit_alpha_t(g - 1)
                    emit_wsum(g - 1)
            emit_alpha_t(NGROUPS - 1)
            emit_wsum(NGROUPS - 1)

    _legalize_waits(nc, mybir)
    return nc


def _get_nc():
    global _cached_nc
    if _cached_nc is None:
        _cached_nc = _build_bass()
    return _cached_nc


def _prep_inputs(hidden_states, W, b, context_vector):
    h = np.ascontiguousarray(np.asarray(hidden_states, dtype=np.float32))
    Wf = np.ascontiguousarray(np.asarray(W, dtype=np.float32))
    bf = np.ascontiguousarray(np.asarray(b, dtype=np.float32).reshape(D))
    cf = np.ascontiguousarray(
        np.asarray(context_vector, dtype=np.float32).reshape(D)
    )
    wt = np.ascontiguousarray(Wf.T)                      # [d, e]
    hr = np.ascontiguousarray(h.reshape(NCORES, T, D))   # per-core natural
    ht = np.ascontiguousarray(hr.transpose(0, 2, 1))     # per-core [D, T]
    return [
        {"x": hr[i], "xt": ht[i], "wt": wt, "bias": bf, "ctxv": cf}
        for i in range(NCORES)
    ]


def run(inputs_kw, **run_kwargs):
    """Run on hardware; returns (BassKernelResults, (wo, aw))."""
    from concourse.bass_utils import run_bass_kernel_spmd

    in_maps = _prep_inputs(**inputs_kw)
    res = run_bass_kernel_spmd(
        _get_nc(), in_maps, core_ids=list(range(NCORES)), **run_kwargs
    )
    wo = np.concatenate([res.results[i]["wo"] for i in range(NCORES)], axis=0)
    aw = np.concatenate([res.results[i]["aw"] for i in range(NCORES)], axis=0)
    return res, (wo, aw)


def kernel(hidden_states, W, b, context_vector):
    _, out = run(
        dict(
            hidden_states=hidden_states,
            W=W,
            b=b,
            context_vector=context_vector,
        )
    )
    return out


# revision 8
# speedup vs baseline: 1.0853x; 1.0853x over previous
"""Trainium2 Bass kernel for attention pooling.

Computation (per reference):
    proj = tanh(h @ W.T + b)            # [B, L, D]
    scores = proj @ ctx                 # [B, L]
    alpha = softmax(scores, axis=L)     # [B, L]
    wo = sum_l alpha[b,l] * h[b,l,:]    # [B, D]
    returns (wo, alpha)

Sharding: data-parallel over batch across 8 NeuronCores (8 batches/core).
Host pre-transposes h and W so the device does zero layout transposes:
  - xt = h_core.T  [D, T]  feeds the projection matmul (contraction dim d
    must live on SBUF partitions for the PE).
  - x  = h_core    [T, D]  natural layout feeds the weighted sum
    (contraction dim l on partitions).

Per-core device pipeline (all static/unrolled, Tile framework):
  proj.T[e,t] accumulated in PSUM over 6 d-chunks (float32r matmuls,
  full PE rate at N=512), tanh+bias fused on ScalarE, scores via M=1
  matmuls (ctx as stationary), softmax on DVE/ACT in 2-batch groups so
  the PE never idles long, weighted sum via M=1 matmuls from natural x.
"""

import numpy as np

B, L, D = 64, 512, 768
NCORES = 8
BLOC = B // NCORES          # 8 batches per core
T = BLOC * L                # 4096 tokens per core
NDT = D // 128              # 6 contraction chunks
NET = D // 128              # 6 output-feature chunks
NL = L // 128               # 4 l-chunks per batch
DH = D // 2                 # 384, n-split for the weighted sum
GROUP = 2                   # batches per softmax group
NGROUPS = BLOC // GROUP

# Main matmul dtype: float32r streams fp32 through the PE at full
# (bf16) rate for free dims >= 256.  Fallback to mybir.dt.float32
# (4 cycles/row) if hardware precision turns out insufficient.
USE_F32R = True

_cached_nc = None


def _legalize_waits(nc, mybir):
    """This walrus build encodes at most ONE semaphore wait per instruction
    (and zero on float32r matmuls, whose waits land on the LDWEIGHTS
    struct). Tile emits correct-but-denser sync_info; split excess waits
    onto same-engine NoOps inserted immediately before the instruction."""
    import bass_rust

    k = 0
    for fn in nc.m.functions:
        for bb in fn.blocks:
            insts = bb.instructions
            out = []
            changed = False
            for inst in insts:
                si = inst.sync_info
                waits = list(si.on_wait) if (si and si.on_wait) else []
                max_waits = 0 if inst.opcode == "Matmult" else 1
                if len(waits) > max_waits:
                    keep = waits[len(waits) - max_waits :] if max_waits else []
                    spill = waits[: len(waits) - max_waits]
                    for w in spill:
                        nop = mybir.InstNoOp(
                            name=f"{inst.name}-w{k}", engine=inst.engine
                        )
                        k += 1
                        nop.sync_info = bass_rust.SyncInfo(
                            on_wait=[w], on_update=[]
                        )
                        out.append(nop)
                    inst.sync_info = bass_rust.SyncInfo(
                        on_wait=keep, on_update=list(si.on_update or [])
                    )
                    changed = True
                out.append(inst)
            if changed:
                bb.instructions = out
    return k


def _build_bass():
    import concourse.bass as bass
    import concourse.mybir as mybir
    import concourse.tile as tile
    from concourse.masks import make_identity

    f32 = mybir.dt.float32
    fr = mybir.dt.float32r if USE_F32R else mybir.dt.float32

    def mm(x):
        return x

    nc = bass.Bass("TRN2", target_bir_lowering=False, debug=False)

    x = nc.dram_tensor("x", [T, D], fr, kind="ExternalInput").ap()
    xt = nc.dram_tensor("xt", [D, T], fr, kind="ExternalInput").ap()
    wt = nc.dram_tensor("wt", [D, D], fr, kind="ExternalInput").ap()
    bias = nc.dram_tensor("bias", [D], f32, kind="ExternalInput").ap()
    ctxv = nc.dram_tensor("ctxv", [D], fr, kind="ExternalInput").ap()
    wo = nc.dram_tensor("wo", [BLOC, D], f32, kind="ExternalOutput").ap()
    aw = nc.dram_tensor("aw", [BLOC, L], f32, kind="ExternalOutput").ap()

    with tile.TileContext(nc) as tc:
        with (
            tc.tile_pool(name="consts", bufs=1) as consts,
            tc.tile_pool(name="xres", bufs=1) as xres,
            tc.tile_pool(name="xtp", bufs=2) as xtp,
            tc.tile_pool(name="projp", bufs=2) as projp,
            tc.tile_pool(name="small", bufs=1) as small,
            tc.tile_pool(name="ppp", bufs=2, space="PSUM") as ppp,
            tc.tile_pool(name="psp", bufs=2, space="PSUM") as psp,
            tc.tile_pool(name="ptp", bufs=1, space="PSUM") as ptp,
            tc.tile_pool(name="pwp", bufs=2, space="PSUM") as pwp,
            tc.tile_pool(name="warmp", bufs=1, space="PSUM") as warmp,
        ):
            # ---- constants ----
            wt_sb = consts.tile([128, NDT, D], fr)
            nc.sync.dma_start(
                out=wt_sb, in_=wt.rearrange("(dt p) e -> p dt e", p=128)
            )
            b_sb = consts.tile([128, NET], f32)
            nc.sync.dma_start(
                out=b_sb, in_=bias.rearrange("(et p) -> p et", p=128)
            )
            c_sb = consts.tile([128, NET], fr)
            nc.sync.dma_start(
                out=c_sb, in_=ctxv.rearrange("(et p) -> p et", p=128)
            )
            ident = consts.tile([128, 128], f32)
            make_identity(nc, ident)

            # Dummy fp32 matmuls on the identity tile: keep the PE busy so
            # the HAM clock-gate reaches/holds K=8/8 across idle windows
            # (input-load startup, softmax tails). fp32 N=512 = ~850ns each.
            warm_ps = warmp.tile([128, 512], f32)
            warm_rhs = consts.tile([128, 512], f32)
            nc.vector.memset(warm_rhs, 1.0)

            def emit_warm(n):
                for _ in range(n):
                    nc.tensor.matmul(
                        warm_ps, lhsT=ident, rhs=warm_rhs, start=True, stop=True
                    )

            emit_warm(10)

            # resident natural-layout activations: token t = (bb*NL + c)*128 + p
            x_sb = xres.tile([128, BLOC, NL, D], fr)
            x_re = x.rearrange("(bb c p) d -> p bb c d", p=128, c=NL)
            xt_re = xt.rearrange("(dt p) (bb t) -> p dt bb t", p=128, bb=BLOC)

            at_sb = small.tile([128, NL, BLOC], fr)   # alpha transposed
            # per-group softmax staging, all based at partition 0
            S_g = [small.tile([GROUP, L], f32, name=f"S_{g}") for g in range(NGROUPS)]
            A_g = [small.tile([GROUP, L], f32, name=f"A_{g}") for g in range(NGROUPS)]
            E_g = [small.tile([GROUP, L], f32, name=f"E_{g}") for g in range(NGROUPS)]
            nmx_g = [small.tile([GROUP, 1], f32, name=f"nmx_{g}") for g in range(NGROUPS)]
            sum_g = [small.tile([GROUP, 1], f32, name=f"sum_{g}") for g in range(NGROUPS)]
            rec_g = [small.tile([GROUP, 1], f32, name=f"rec_{g}") for g in range(NGROUPS)]

            tanh = mybir.ActivationFunctionType.Tanh
            exp = mybir.ActivationFunctionType.Exp

            def emit_batch(bb):
                g, j = divmod(bb, GROUP)
                xt_t = xtp.tile([128, NDT, L], fr, tag="xt", name=f"xt_{bb}")
                nc.sync.dma_start(out=xt_t, in_=xt_re[:, :, bb, :])

                proj = projp.tile([128, NET, L], fr, tag="proj", name=f"proj_{bb}")
                for et in range(NET):
                    pp = ppp.tile([128, L], f32, tag="pp", name=f"pp_{bb}_{et}")
                    for dt in range(NDT):
                        nc.tensor.matmul(
                            pp,
                            lhsT=mm(wt_sb[:, dt, et * 128 : (et + 1) * 128]),
                            rhs=mm(xt_t[:, dt, :]),
                            start=(dt == 0),
                            stop=(dt == NDT - 1),
                        )
                    nc.scalar.activation(
                        out=proj[:, et, :],
                        in_=pp,
                        func=tanh,
                        bias=b_sb[:, et : et + 1],
                        scale=1.0,
                    )
                ps = psp.tile([1, L], f32, tag="ps", name=f"ps_{bb}")
                for et in range(NET):
                    nc.tensor.matmul(
                        ps,
                        lhsT=mm(c_sb[:, et : et + 1]),
                        rhs=mm(proj[:, et, :]),
                        start=(et == 0),
                        stop=(et == NET - 1),
                    )
                # PSUM has no DMA port: drain scores via DVE, then DMA the
                # row to row j of the group staging tile (DMA can retarget
                # partitions; compute engines cannot).
                srow = small.tile([1, L], f32, tag="srow", name=f"srow_{bb}", bufs=2)
                nc.vector.tensor_copy(srow, ps)
                nc.sync.dma_start(out=S_g[g][j : j + 1, :], in_=srow)
                # natural-layout load for the weighted sum, deferred so it
                # never delays the next batch's xt on the DMA queues
                nc.sync.dma_start(out=x_sb[:, bb, :, :], in_=x_re[:, bb, :, :])

            def emit_softmax(g):
                nc.vector.reduce_max(
                    out=nmx_g[g],
                    in_=S_g[g],
                    axis=mybir.AxisListType.X,
                    negate=True,
                )
                nc.scalar.activation(
                    out=E_g[g],
                    in_=S_g[g],
                    func=exp,
                    bias=nmx_g[g],
                    scale=1.0,
                    accum_out=sum_g[g],
                )
                nc.vector.reciprocal(rec_g[g], sum_g[g])
                nc.vector.tensor_scalar_mul(A_g[g], E_g[g], rec_g[g])
                nc.sync.dma_start(
                    out=aw[g * GROUP : (g + 1) * GROUP, :], in_=A_g[g]
                )

            def emit_alpha_t(g):
                # PE work that depends on softmax(g): emitted once group g+1's
                # projection work is already queued so the PE never stalls.
                lo = g * GROUP
                for c in range(NL):
                    pt = ptp.tile([128, GROUP], f32, tag="pt", name=f"pt_{g}_{c}")
                    nc.tensor.transpose(
                        pt,
                        A_g[g][:, c * 128 : (c + 1) * 128],
                        ident[:GROUP, :GROUP],
                    )
                    nc.vector.tensor_copy(at_sb[:, c, lo : lo + GROUP], pt)

            def emit_wsum(g):
                for bb in range(g * GROUP, (g + 1) * GROUP):
                    wrow = small.tile([1, D], f32, tag="wrow", name=f"wrow_{bb}", bufs=2)
                    for h in range(2):
                        pw = pwp.tile([1, DH], f32, tag="pw", name=f"pw_{bb}_{h}")
                        for c in range(NL):
                            nc.tensor.matmul(
                                pw,
                                lhsT=mm(at_sb[:, c, bb : bb + 1]),
                                rhs=mm(x_sb[:, bb, c, h * DH : (h + 1) * DH]),
                                start=(c == 0),
                                stop=(c == NL - 1),
                            )
                        nc.vector.tensor_copy(wrow[:, h * DH : (h + 1) * DH], pw)
                    nc.sync.dma_start(out=wo[bb : bb + 1, :], in_=wrow)

            for g in range(NGROUPS):
                for bb in range(g * GROUP, (g + 1) * GROUP):
                    emit_batch(bb)
                emit_softmax(g)
                if g > 0:
                    emit_alpha_t(g - 1)
                    emit_wsum(g - 1)
            emit_warm(4)
            emit_alpha_t(NGROUPS - 1)
            emit_wsum(NGROUPS - 1)

    _legalize_waits(nc, mybir)
    return nc


def _get_nc():
    global _cached_nc
    if _cached_nc is None:
        _cached_nc = _build_bass()
    return _cached_nc


def _prep_inputs(hidden_states, W, b, context_vector):
    h = np.ascontiguousarray(np.asarray(hidden_states, dtype=np.float32))
    Wf = np.ascontiguousarray(np.asarray(W, dtype=np.float32))
    bf = np.ascontiguousarray(np.asarray(b, dtype=np.float32).reshape(D))
    cf = np.ascontiguousarray(
        np.asarray(context_vector, dtype=np.float32).reshape(D)
    )
    wt = np.ascontiguousarray(Wf.T)                      # [d, e]
    hr = np.ascontiguousarray(h.reshape(NCORES, T, D))   # per-core natural
    ht = np.ascontiguousarray(hr.transpose(0, 2, 1))     # per-core [D, T]
    return [
        {"x": hr[i], "xt": ht[i], "wt": wt, "bias": bf, "ctxv": cf}
        for i in range(NCORES)
    ]


def run(inputs_kw, **run_kwargs):
    """Run on hardware; returns (BassKernelResults, (wo, aw))."""
    from concourse.bass_utils import run_bass_kernel_spmd

    in_maps = _prep_inputs(**inputs_kw)
    res = run_bass_kernel_spmd(
        _get_nc(), in_maps, core_ids=list(range(NCORES)), **run_kwargs
    )
    wo = np.concatenate([res.results[i]["wo"] for i in range(NCORES)], axis=0)
    aw = np.concatenate([res.results[i]["aw"] for i in range(NCORES)], axis=0)
    return res, (wo, aw)


def kernel(hidden_states, W, b, context_vector):
    _, out = run(
        dict(
            hidden_states=hidden_states,
            W=W,
            b=b,
            context_vector=context_vector,
        )
    )
    return out
